# revision 7
# baseline (speedup 1.0000x reference)
"""Trainium2 Bass kernel for nn_CustomLoss_17875653886357.

Contrastive-style loss vs. the last row (anchor) of the batch:
    lab  = (labels != labels[-1])                        [N]
    dist = ||coords - coords[-1]||^2                     [N]
    loss = sum((1-lab)*dist + lab*max(0, MARGIN-dist))   scalar

Sharding: data-parallel over N across 8 NeuronCores (4096 rows each).
The anchor row (3 floats + 1 int) is baked into the compiled kernel as
immediates; each core produces a scalar partial sum; host adds the 8
partials (the gather/unshard step).
"""

from contextlib import ExitStack

import numpy as np

import concourse.bass as bass
import concourse.mybir as mybir
import concourse.tile as tile
from concourse import bacc
from concourse.bass_utils import run_bass_kernel_spmd

N, D = 32768, 3
NCORES = 8
NS = N // NCORES  # rows per core = 4096
P = 128  # SBUF partitions
M = NS // P  # rows per partition = 32
MARGIN = 500.0

F32 = mybir.dt.float32
I32 = mybir.dt.int32
Alu = mybir.AluOpType


def _build(anchor_pt, anchor_lab):
    """Build the per-core Bass program. Anchor values are compile-time
    immediates (the kernel is compiled per call, after inputs are known)."""
    ax, ay, az = (float(v) for v in anchor_pt)
    al = float(int(anchor_lab))  # labels are small ints; exact in f32

    nc = bacc.Bacc("TRN2", target_bir_lowering=False, debug=False)
    coords_d = nc.declare_dram_parameter("coords", [P, M * D], F32, isOutput=False)
    labels_d = nc.declare_dram_parameter("labels", [P, M], I32, isOutput=False)
    out_d = nc.declare_dram_parameter("out", [1, 1], F32, isOutput=True)

    with tile.TileContext(nc) as tc, ExitStack() as ctx:
        pool = ctx.enter_context(tc.tile_pool(name="sbuf", bufs=1))
        psum = ctx.enter_context(tc.tile_pool(name="psum", bufs=1, space="PSUM"))

        C = pool.tile([P, M * D], F32)  # coords, 32 rows x (x,y,z) per partition
        L = pool.tile([P, M], I32)
        nc.sync.dma_start(C[:], coords_d[:])
        nc.sync.dma_start(L[:], labels_d[:])

        # Anchor broadcast tile: AB[p, 3m+k] = a_k (on DVE: same engine as
        # the consumer, so ordering is program order — no extra sem waits,
        # which overflow the per-instruction sync-wait limit in codegen)
        AB = pool.tile([P, M * D], F32)
        AB3 = AB[:].rearrange("p (m d) -> p m d", d=D)
        nc.vector.memset(AB3[:, :, 0], ax)
        nc.vector.memset(AB3[:, :, 1], ay)
        nc.vector.memset(AB3[:, :, 2], az)
        ONES = pool.tile([P, 1], F32)
        nc.vector.memset(ONES[:], 1.0)

        # e = (label == anchor_label) as f32 0/1
        Lf = pool.tile([P, M], F32)
        nc.vector.tensor_copy(Lf[:], L[:])  # i32 -> f32 (values < 100, exact)
        E = pool.tile([P, M], F32)
        nc.vector.tensor_scalar(E[:], Lf[:], al, None, Alu.is_equal)

        # DN = -dist = -sum((c - a)^2 over 3 comps)
        DIFF = pool.tile([P, M * D], F32)
        nc.vector.tensor_sub(DIFF[:], C[:], AB[:])
        SQ = pool.tile([P, M * D], F32)
        nc.vector.tensor_tensor(SQ[:], DIFF[:], DIFF[:], Alu.mult)
        DN = pool.tile([P, M], F32)
        SQ3 = SQ[:].rearrange("p (m d) -> p m d", d=D)
        nc.vector.tensor_reduce(
            DN[:], SQ3, axis=mybir.AxisListType.X, op=Alu.add, negate=True
        )

        # H = max(MARGIN - dist, 0) = max(DN + MARGIN, 0)
        H = pool.tile([P, M], F32)
        nc.vector.tensor_scalar(H[:], DN[:], MARGIN, 0.0, Alu.add, Alu.max)
        # B = H - dist;  EM = e*B;  loss = H - EM;  RS = per-partition sum(loss)
        B = pool.tile([P, M], F32)
        nc.vector.tensor_add(B[:], DN[:], H[:])
        EM = pool.tile([P, M], F32)
        nc.vector.tensor_tensor(EM[:], E[:], B[:], Alu.mult)
        LOSS = pool.tile([P, M], F32)
        RS = pool.tile([P, 1], F32)
        nc.vector.scalar_tensor_tensor(
            LOSS[:], EM[:], -1.0, H[:], Alu.mult, Alu.add, accum_out=RS[:]
        )

        # Cross-partition reduction: [1,1] = RS.T @ ones
        ACC = psum.tile([1, 1], F32)
        nc.tensor.matmul(ACC[:], RS[:], ONES[:], start=True, stop=True)
        OUT = pool.tile([1, 1], F32)
        nc.vector.tensor_copy(OUT[:], ACC[:])
        nc.sync.dma_start(out_d[:], OUT[:])

    nc.compile()
    return nc


def build_nc_and_inmaps(batched_labels, batched_predicted_coords):
    labels = np.ascontiguousarray(batched_labels)
    coords = np.ascontiguousarray(batched_predicted_coords, dtype=np.float32)
    assert labels.shape == (N,) and coords.shape == (N, D)
    if labels.dtype != np.int32:
        labels = labels.astype(np.int32)

    nc = _build(coords[-1], labels[-1])

    in_maps = []
    for i in range(NCORES):
        sl = slice(i * NS, (i + 1) * NS)
        in_maps.append(
            {
                "coords": np.ascontiguousarray(coords[sl]).reshape(P, M * D),
                "labels": np.ascontiguousarray(labels[sl]).reshape(P, M),
            }
        )
    return nc, in_maps


def kernel(batched_labels, batched_predicted_coords, _trace=False, _results=[None]):
    nc, in_maps = build_nc_and_inmaps(batched_labels, batched_predicted_coords)
    res = run_bass_kernel_spmd(nc, in_maps, core_ids=list(range(NCORES)), trace=_trace)
    _results[0] = res
    total = np.float64(0.0)
    for r in res.results:
        total += np.float64(r["out"][0, 0])
    return np.array(np.float32(total))
